# revision 47
# baseline (speedup 1.0000x reference)
"""Bahdanau attention on 8 Trainium2 NeuronCores.

Strategy: pure data parallel over the batch dim (4 batches per core).
Host pre-transposes encoder_outputs to [B, E, S] bf16 so the big
projection matmul needs no on-device transpose. Per core:

  projT[a, s] = sum_e W_h[e, a] * enc[s, e]        (PE, bf16, psum f32)
  t = tanh(projT + dec_proj[a])                    (ACT, fused bias)
  scores[s]  = sum_a v[a] * t[a, s]                (PE, v as lhsT, M=1)
  p = exp(scores) * mask                           (ACT exp + DVE mult)
  weights = p / sum(p)                             (DVE)
  context[e] = sum_s p[s] * encT[e, s] / sum(p)    (PE bcast + DVE/ACT)

No max-subtraction is needed in the softmax: |scores| <= ||v||_1 ~ 18,
exp() is safely in fp32 range.

The batch loop is software-pipelined one deep: the context reduction
of batch b-1 (p-broadcast matmul, multiply, reduce) is interleaved
into batch b's score emission per s-tile, so PE never stalls on the
softmax and the DVE/ACT context work rides under the next batch's
matmuls. Context free-dim reductions alternate DVE/ACT to balance
engine load.
"""

import contextlib
import ctypes
import os
import sys
import types

import numpy as np
import ml_dtypes

B, S, EH, AT = 32, 4096, 1024, 512
NCORES = 8
BL = B // NCORES  # 4 batches per core
EC = EH // 128    # 8 e-chunks
AC = AT // 128    # 4 a-chunks
ST = S // 512     # 8 s-tiles

BF16 = ml_dtypes.bfloat16


def _install_axon_hooks_shim():
    """concourse.bass_utils imports antenv.axon_hooks when tracing is
    requested; some images lack that module. Provide an equivalent."""
    try:
        import antenv.axon_hooks  # noqa: F401
        return
    except ImportError:
        pass

    def _make_hook():
        so_path = "/opt/axon/libaxon_pjrt.so"
        if not os.path.exists(so_path):
            return None
        lib = ctypes.CDLL(so_path)
        if not hasattr(lib, "axon_start_nrt_profile"):
            return None
        lib.axon_start_nrt_profile.argtypes = [
            ctypes.POINTER(ctypes.c_int64),
            ctypes.c_size_t,
        ]
        lib.axon_start_nrt_profile.restype = ctypes.c_int64
        lib.axon_stop_nrt_profile.argtypes = [ctypes.c_char_p]
        lib.axon_stop_nrt_profile.restype = ctypes.c_int64

        @contextlib.contextmanager
        def _hook(output_dir, device_ids):
            import jax

            jax.devices()
            if device_ids:
                ids = (ctypes.c_int64 * len(device_ids))(*device_ids)
                rc = lib.axon_start_nrt_profile(ids, len(device_ids))
            else:
                rc = lib.axon_start_nrt_profile(None, 0)
            if rc != 0:
                raise RuntimeError(f"axon_start_nrt_profile rc={rc}")
            try:
                yield
            finally:
                n = lib.axon_stop_nrt_profile(str(output_dir).encode())
                if n < 0:
                    raise RuntimeError(f"axon_stop_nrt_profile rc={n}")

        return _hook

    mod = types.ModuleType("antenv.axon_hooks")
    _state = {"hook": _make_hook()}
    mod.set_axon_ntff_profile_hook = lambda h: _state.__setitem__("hook", h)
    mod.get_axon_ntff_profile_hook = lambda: _state["hook"]
    sys.modules["antenv.axon_hooks"] = mod


_NC_CACHE = None


def _build():
    global _NC_CACHE
    if _NC_CACHE is not None:
        return _NC_CACHE

    _install_axon_hooks_shim()
    import dataclasses
    import concourse.tile as tile
    from concourse import bacc, mybir

    f32 = mybir.dt.float32
    bf16 = mybir.dt.bfloat16
    Act = mybir.ActivationFunctionType
    Alu = mybir.AluOpType
    AX = mybir.AxisListType

    nc = bacc.Bacc("TRN2", target_bir_lowering=False, debug=False,
                   num_devices=NCORES)

    encT = nc.declare_dram_parameter("encT", [BL, EC, 128, S], bf16,
                                     isOutput=False)
    wh = nc.declare_dram_parameter("wh", [EH, AT], bf16, isOutput=False)
    ws = nc.declare_dram_parameter("ws", [EH, AT], bf16, isOutput=False)
    dsT = nc.declare_dram_parameter("dsT", [EH, BL], bf16, isOutput=False)
    vt = nc.declare_dram_parameter("vt", [128, AC], bf16, isOutput=False)
    maskb = nc.declare_dram_parameter("maskb", [BL, S], bf16, isOutput=False)
    ident = nc.declare_dram_parameter("ident", [128, 128], f32,
                                      isOutput=False)
    context = nc.declare_dram_parameter("context", [BL, EH], f32,
                                        isOutput=True)
    weights = nc.declare_dram_parameter("weights", [BL, S], f32,
                                        isOutput=True)

    with tile.TileContext(nc) as tc, contextlib.ExitStack() as ctx:
        const = ctx.enter_context(tc.tile_pool(name="const", bufs=1))
        encp = ctx.enter_context(tc.tile_pool(name="encp", bufs=2))
        tp = ctx.enter_context(tc.tile_pool(name="tp", bufs=1))
        psA = ctx.enter_context(tc.tile_pool(name="psA", bufs=4, space="PSUM"))
        psS = ctx.enter_context(tc.tile_pool(name="psS", bufs=2, space="PSUM"))
        psB = ctx.enter_context(tc.tile_pool(name="psB", bufs=2, space="PSUM"))

        # --- constants / small inputs ---
        wh_sb = const.tile([128, EC, AT], bf16)
        dsT_sb = const.tile([128, EC, BL], bf16)
        nc.sync.dma_start(dsT_sb[:], dsT.ap().rearrange("(c p) b -> p c b", p=128))
        # W_s borrows an enc slot (only needed during the prologue)
        ws_sb = encp.tile([128, EC, AT], bf16, tag="enc")
        nc.scalar.dma_start(ws_sb[:], ws.ap().rearrange("(c p) a -> p c a", p=128))
        v_sb = const.tile([128, AC], bf16)

        bias_sb = const.tile([128, AC * BL], f32)
        ctx_cols = const.tile([128, BL * EC], f32)
        r_cols = const.tile([1, BL], f32)
        d_cols = const.tile([1, BL], f32)
        ones_bf = const.tile([1, 128], bf16)
        nc.vector.memset(ones_bf[:], 1.0)
        ones_f32 = const.tile([1, 128], f32)
        nc.vector.memset(ones_f32[:], 1.0)
        ident_sb = const.tile([128, 128], f32)

        # --- per-batch state (pipelined one deep) ---
        enc_tiles = [None] * BL
        p_rows = [None] * BL
        mask_rows = [None] * BL
        pbf_rows = [None] * BL
        ctx_sts = [None] * BL

        def bcast3(ap):
            # [128, 512] -> [128, EC, 512] with a zero-stride middle dim
            return dataclasses.replace(
                ap, ap=[ap.ap[0], [0, EC], ap.ap[1]])

        def emit_load(b, after_first=None):
            enc_sb = encp.tile([128, EC, S], bf16, tag="enc", name=f"enc_{b}")
            enc_tiles[b] = enc_sb
            for st in range(ST):
                nc.sync.dma_start(
                    enc_sb[:, :, st * 512:(st + 1) * 512],
                    encT.ap()[b, :, :, st * 512:(st + 1) * 512]
                    .rearrange("c p s -> p c s"))
                if st == 0 and after_first is not None:
                    after_first()

        def emit_scores_tile(b, st):
            ps = psS.tile([1, 512], f32, tag="ps", name=f"ps_{b}_{st}")
            for ac in range(AC):
                pa = psA.tile([128, 512], f32, tag="pa",
                              name=f"pa_{b}_{st}_{ac}")
                for ec in range(EC):
                    nc.tensor.matmul(
                        pa[:],
                        wh_sb[:, ec, ac * 128:(ac + 1) * 128],
                        enc_tiles[b][:, ec, st * 512:(st + 1) * 512],
                        start=(ec == 0), stop=(ec == EC - 1),
                    )
                tt = tp.tile([128, 512], bf16, tag="tt",
                             name=f"tt_{b}_{st}_{ac}", bufs=3)
                nc.scalar.activation(
                    tt[:], pa[:], Act.Tanh,
                    bias=bias_sb[:, ac * BL + b:ac * BL + b + 1],
                )
                nc.tensor.matmul(
                    ps[:], v_sb[:, ac:ac + 1], tt[:],
                    start=(ac == 0), stop=(ac == AC - 1),
                )
            nc.scalar.activation(
                p_rows[b][:, st * 512:(st + 1) * 512], ps[:], Act.Exp)
            # mask + bf16 cast for this s-slice, so the context tile for
            # this slice can start without waiting for the whole batch
            sl = slice(st * 512, (st + 1) * 512)
            nc.vector.tensor_tensor(p_rows[b][:, sl], p_rows[b][:, sl],
                                    mask_rows[b][:, sl], Alu.mult)
            nc.vector.tensor_copy(pbf_rows[b][:, sl], p_rows[b][:, sl])

        def emit_softmax(b):
            p_row = p_rows[b]
            nc.vector.tensor_reduce(d_cols[:, b:b + 1], p_row[:], AX.X,
                                    Alu.add)
            nc.vector.reciprocal(r_cols[:, b:b + 1], d_cols[:, b:b + 1])
            # normalize in place and write the weights row out
            nc.vector.tensor_scalar_mul(p_row[:], p_row[:],
                                        r_cols[:, b:b + 1])
            nc.sync.dma_start(weights.ap()[b:b + 1, :], p_row[:])

        def emit_ctx_tile(b, st):
            # broadcast p[st] to 128 partitions via K=1 ones-matmul
            pbc = psB.tile([128, 512], f32, tag="pbc", name=f"pbc_{b}_{st}")
            nc.tensor.matmul(pbc[:], ones_bf[:],
                             pbf_rows[b][:, st * 512:(st + 1) * 512],
                             start=True, stop=True)
            pbc_sb = tp.tile([128, 512], bf16, tag="pbcsb",
                             name=f"pbcsb_{b}_{st}", bufs=2)
            nc.scalar.copy(pbc_sb[:], pbc[:])
            scr = tp.tile([128, EC, 512], bf16, tag="scr",
                          name=f"scr_{b}_{st}", bufs=2)
            nc.vector.tensor_tensor(
                scr[:], enc_tiles[b][:, :, st * 512:(st + 1) * 512],
                bcast3(pbc_sb[:]), Alu.mult)
            # partial sums over this s-slice: [128, EC, 512] -> [128, EC]
            dst = ctx_sts[b][:, st * EC:(st + 1) * EC]
            if b == BL - 1 and st >= ST - 2:
                # drain phase: split the reduce across both engines
                nc.vector.tensor_reduce(
                    ctx_sts[b][:, st * EC:st * EC + EC // 2],
                    scr[:, :EC // 2, :], AX.X, Alu.add)
                for ec in range(EC // 2, EC):
                    nc.scalar.activation(
                        scr[:, ec, :], scr[:, ec, :], Act.Copy,
                        accum_out=ctx_sts[b][:, st * EC + ec:st * EC + ec + 1])
            elif st % 4 != 3:
                nc.vector.tensor_reduce(dst, scr[:], AX.X, Alu.add)
            else:
                for ec in range(EC):
                    nc.scalar.activation(
                        scr[:, ec, :], scr[:, ec, :], Act.Copy,
                        accum_out=ctx_sts[b][:, st * EC + ec:st * EC + ec + 1])

        def emit_ctx_final(b):
            # sum partials over s-tiles: view [128, ST, EC] as [128, EC, ST]
            nc.vector.tensor_reduce(
                ctx_cols[:, b * EC:(b + 1) * EC],
                ctx_sts[b].rearrange("p (s e) -> p e s", e=EC),
                AX.X, Alu.add)

        def begin_batch(b, after_first=None):
            emit_load(b, after_first)
            mask_row = tp.tile([1, S], bf16, tag="mask", name=f"mask_{b}",
                               bufs=1)
            nc.sync.dma_start(mask_row[:], maskb.ap()[b:b + 1, :])
            p_row = tp.tile([1, S], f32, tag="prow", name=f"p_{b}", bufs=2)
            p_rows[b] = p_row
            mask_rows[b] = mask_row
            pbf_rows[b] = tp.tile([1, S], bf16, tag="pbf", name=f"pbf_{b}",
                                  bufs=1)
            ctx_sts[b] = tp.tile([128, ST * EC], f32, tag="ctxst",
                                 name=f"ctxst_{b}", bufs=2)

        # --- main loop, software-pipelined one s-tile deep ---
        # flat list of (b, st) work items; ctx lags scores by one item
        items = [(b, st) for b in range(BL) for st in range(ST)]
        nc.sync.dma_start(v_sb[:], vt.ap())
        begin_batch(0, after_first=lambda: nc.sync.dma_start(
            wh_sb[:], wh.ap().rearrange("(c p) a -> p c a", p=128)))
        nc.sync.dma_start(ident_sb[:], ident.ap())
        # decoder projection: bias[a, b] = sum_d W_s[d, a] * ds[b, d]
        pd = psB.tile([128, AC * BL], f32, tag="pbc", name="pd")
        for ac in range(AC):
            for ec in range(EC):
                nc.tensor.matmul(
                    pd[:, ac * BL:(ac + 1) * BL],
                    ws_sb[:, ec, ac * 128:(ac + 1) * 128],
                    dsT_sb[:, ec, :],
                    start=(ec == 0), stop=(ec == EC - 1),
                )
        nc.scalar.copy(bias_sb[:], pd[:])
        for i, (b, st) in enumerate(items):
            emit_scores_tile(b, st)
            if st == 0 and b + 1 < BL:
                begin_batch(b + 1)
            if i > 0:
                pb, pst = items[i - 1]
                emit_ctx_tile(pb, pst)
                if pst == ST - 1:
                    emit_ctx_final(pb)
                    emit_softmax(pb)
        lb, lst = items[-1]
        emit_softmax(lb)
        emit_ctx_tile(lb, lst)
        emit_ctx_final(lb)

        # --- normalize context, transpose on PE, write out contiguously ---
        pr = psB.tile([128, BL], f32, tag="pbc", name="pr")
        nc.tensor.matmul(pr[:], ones_f32[:], r_cols[:], start=True, stop=True)
        rrep = tp.tile([128, BL], f32, tag="rrep", name="rrep", bufs=1)
        nc.scalar.copy(rrep[:], pr[:])
        ctx_fin = tp.tile([128, BL * EC], f32, tag="ctxfin", name="ctx_fin",
                          bufs=1)
        for b in range(BL):
            nc.vector.tensor_scalar_mul(ctx_fin[:, b * EC:(b + 1) * EC],
                                        ctx_cols[:, b * EC:(b + 1) * EC],
                                        rrep[:, b:b + 1])
        ctxT_ps = psA.tile([BL * EC, 128], f32, tag="pa", name="ctxT_ps")
        nc.tensor.transpose(ctxT_ps[:], ctx_fin[:], ident_sb[:])
        ctxT_sb = tp.tile([BL * EC, 128], f32, tag="ctxT", name="ctxT_sb",
                          bufs=1)
        nc.scalar.copy(ctxT_sb[:], ctxT_ps[:])
        nc.sync.dma_start(
            context.ap().rearrange("b (c p) -> (b c) p", p=128), ctxT_sb[:])

    nc.compile()
    _NC_CACHE = nc
    return nc


def kernel(decoder_state, encoder_outputs, mask, W_h, W_s, v):
    nc = _build()
    from concourse.bass_utils import run_bass_kernel_spmd

    enc_bf = np.asarray(encoder_outputs, dtype=np.float32).astype(BF16)
    # [B, S, E] -> [B, E, S] -> [B, EC, 128, S]
    encT_all = np.ascontiguousarray(enc_bf.transpose(0, 2, 1)).reshape(
        B, EC, 128, S)
    wh_bf = np.asarray(W_h, dtype=np.float32).astype(BF16)
    ws_bf = np.asarray(W_s, dtype=np.float32).astype(BF16)
    ds = np.asarray(decoder_state, dtype=np.float32)
    v_t = np.ascontiguousarray(
        np.asarray(v, dtype=np.float32).reshape(AC, 128).T).astype(BF16)
    mask_bf = np.asarray(mask).astype(BF16)
    ident = np.eye(128, dtype=np.float32)

    in_maps = []
    for i in range(NCORES):
        sl = slice(i * BL, (i + 1) * BL)
        in_maps.append({
            "encT": encT_all[sl],
            "wh": wh_bf,
            "ws": ws_bf,
            "dsT": np.ascontiguousarray(ds[sl].T).astype(BF16),
            "vt": v_t,
            "maskb": mask_bf[sl],
            "ident": ident,
        })

    res = run_bass_kernel_spmd(nc, in_maps, core_ids=list(range(NCORES)))
    ctx = np.concatenate(
        [np.asarray(res.results[i]["context"]) for i in range(NCORES)], axis=0)
    wts = np.concatenate(
        [np.asarray(res.results[i]["weights"]) for i in range(NCORES)], axis=0)
    return ctx.astype(np.float32), wts.astype(np.float32)


# revision 48
# speedup vs baseline: 1.0135x; 1.0135x over previous
"""Bahdanau attention on 8 Trainium2 NeuronCores.

Strategy: pure data parallel over the batch dim (4 batches per core).
Host pre-transposes encoder_outputs to [B, E, S] bf16 so the big
projection matmul needs no on-device transpose. Per core:

  projT[a, s] = sum_e W_h[e, a] * enc[s, e]        (PE, bf16, psum f32)
  t = tanh(projT + dec_proj[a])                    (ACT, fused bias)
  scores[s]  = sum_a v[a] * t[a, s]                (PE, v as lhsT, M=1)
  p = exp(scores) * mask                           (ACT exp + DVE mult)
  weights = p / sum(p)                             (DVE)
  context[e] = sum_s p[s] * encT[e, s] / sum(p)    (PE bcast + DVE/ACT)

No max-subtraction is needed in the softmax: |scores| <= ||v||_1 ~ 18,
exp() is safely in fp32 range.

The batch loop is software-pipelined one deep: the context reduction
of batch b-1 (p-broadcast matmul, multiply, reduce) is interleaved
into batch b's score emission per s-tile, so PE never stalls on the
softmax and the DVE/ACT context work rides under the next batch's
matmuls. Context free-dim reductions alternate DVE/ACT to balance
engine load.
"""

import contextlib
import ctypes
import os
import sys
import types

import numpy as np
import ml_dtypes

B, S, EH, AT = 32, 4096, 1024, 512
NCORES = 8
BL = B // NCORES  # 4 batches per core
EC = EH // 128    # 8 e-chunks
AC = AT // 128    # 4 a-chunks
ST = S // 512     # 8 s-tiles

BF16 = ml_dtypes.bfloat16


def _install_axon_hooks_shim():
    """concourse.bass_utils imports antenv.axon_hooks when tracing is
    requested; some images lack that module. Provide an equivalent."""
    try:
        import antenv.axon_hooks  # noqa: F401
        return
    except ImportError:
        pass

    def _make_hook():
        so_path = "/opt/axon/libaxon_pjrt.so"
        if not os.path.exists(so_path):
            return None
        lib = ctypes.CDLL(so_path)
        if not hasattr(lib, "axon_start_nrt_profile"):
            return None
        lib.axon_start_nrt_profile.argtypes = [
            ctypes.POINTER(ctypes.c_int64),
            ctypes.c_size_t,
        ]
        lib.axon_start_nrt_profile.restype = ctypes.c_int64
        lib.axon_stop_nrt_profile.argtypes = [ctypes.c_char_p]
        lib.axon_stop_nrt_profile.restype = ctypes.c_int64

        @contextlib.contextmanager
        def _hook(output_dir, device_ids):
            import jax

            jax.devices()
            if device_ids:
                ids = (ctypes.c_int64 * len(device_ids))(*device_ids)
                rc = lib.axon_start_nrt_profile(ids, len(device_ids))
            else:
                rc = lib.axon_start_nrt_profile(None, 0)
            if rc != 0:
                raise RuntimeError(f"axon_start_nrt_profile rc={rc}")
            try:
                yield
            finally:
                n = lib.axon_stop_nrt_profile(str(output_dir).encode())
                if n < 0:
                    raise RuntimeError(f"axon_stop_nrt_profile rc={n}")

        return _hook

    mod = types.ModuleType("antenv.axon_hooks")
    _state = {"hook": _make_hook()}
    mod.set_axon_ntff_profile_hook = lambda h: _state.__setitem__("hook", h)
    mod.get_axon_ntff_profile_hook = lambda: _state["hook"]
    sys.modules["antenv.axon_hooks"] = mod


_NC_CACHE = None


def _build():
    global _NC_CACHE
    if _NC_CACHE is not None:
        return _NC_CACHE

    _install_axon_hooks_shim()
    import dataclasses
    import concourse.tile as tile
    from concourse import bacc, mybir

    f32 = mybir.dt.float32
    bf16 = mybir.dt.bfloat16
    Act = mybir.ActivationFunctionType
    Alu = mybir.AluOpType
    AX = mybir.AxisListType

    nc = bacc.Bacc("TRN2", target_bir_lowering=False, debug=False,
                   num_devices=NCORES)

    encT = nc.declare_dram_parameter("encT", [BL, EC, 128, S], bf16,
                                     isOutput=False)
    wh = nc.declare_dram_parameter("wh", [EH, AT], bf16, isOutput=False)
    ws = nc.declare_dram_parameter("ws", [EH, AT], bf16, isOutput=False)
    dsT = nc.declare_dram_parameter("dsT", [EH, BL], bf16, isOutput=False)
    vt = nc.declare_dram_parameter("vt", [128, AC], bf16, isOutput=False)
    maskb = nc.declare_dram_parameter("maskb", [BL, S], bf16, isOutput=False)
    ident = nc.declare_dram_parameter("ident", [128, 128], f32,
                                      isOutput=False)
    context = nc.declare_dram_parameter("context", [BL, EH], f32,
                                        isOutput=True)
    weights = nc.declare_dram_parameter("weights", [BL, S], f32,
                                        isOutput=True)

    with tile.TileContext(nc) as tc, contextlib.ExitStack() as ctx:
        const = ctx.enter_context(tc.tile_pool(name="const", bufs=1))
        encp = ctx.enter_context(tc.tile_pool(name="encp", bufs=2))
        tp = ctx.enter_context(tc.tile_pool(name="tp", bufs=1))
        psA = ctx.enter_context(tc.tile_pool(name="psA", bufs=4, space="PSUM"))
        psS = ctx.enter_context(tc.tile_pool(name="psS", bufs=2, space="PSUM"))
        psB = ctx.enter_context(tc.tile_pool(name="psB", bufs=2, space="PSUM"))

        # --- constants / small inputs ---
        wh_sb = const.tile([128, EC, AT], bf16)
        dsT_sb = const.tile([128, EC, BL], bf16)
        nc.sync.dma_start(dsT_sb[:], dsT.ap().rearrange("(c p) b -> p c b", p=128))
        # W_s borrows an enc slot (only needed during the prologue)
        ws_sb = encp.tile([128, EC, AT], bf16, tag="enc")
        nc.scalar.dma_start(ws_sb[:], ws.ap().rearrange("(c p) a -> p c a", p=128))
        v_sb = const.tile([128, AC], bf16)

        bias_sb = const.tile([128, AC * BL], f32)
        ctx_cols = const.tile([128, BL * EC], f32)
        r_cols = const.tile([1, BL], f32)
        d_cols = const.tile([1, BL], f32)
        ones_bf = const.tile([1, 128], bf16)
        nc.vector.memset(ones_bf[:], 1.0)
        ones_f32 = const.tile([1, 128], f32)
        nc.vector.memset(ones_f32[:], 1.0)
        ident_sb = const.tile([128, 128], f32)

        # --- per-batch state (pipelined one deep) ---
        enc_tiles = [None] * BL
        p_rows = [None] * BL
        mask_rows = [None] * BL
        pbf_rows = [None] * BL
        ctx_sts = [None] * BL

        def bcast3(ap):
            # [128, 512] -> [128, EC, 512] with a zero-stride middle dim
            return dataclasses.replace(
                ap, ap=[ap.ap[0], [0, EC], ap.ap[1]])

        def emit_load(b, after_first=None):
            enc_sb = encp.tile([128, EC, S], bf16, tag="enc", name=f"enc_{b}")
            enc_tiles[b] = enc_sb
            for st in range(ST):
                nc.sync.dma_start(
                    enc_sb[:, :, st * 512:(st + 1) * 512],
                    encT.ap()[b, :, :, st * 512:(st + 1) * 512]
                    .rearrange("c p s -> p c s"))
                if st == 0 and after_first is not None:
                    after_first()

        def emit_scores_tile(b, st):
            ps = psS.tile([1, 512], f32, tag="ps", name=f"ps_{b}_{st}")
            for ac in range(AC):
                pa = psA.tile([128, 512], f32, tag="pa",
                              name=f"pa_{b}_{st}_{ac}")
                for ec in range(EC):
                    nc.tensor.matmul(
                        pa[:],
                        wh_sb[:, ec, ac * 128:(ac + 1) * 128],
                        enc_tiles[b][:, ec, st * 512:(st + 1) * 512],
                        start=(ec == 0), stop=(ec == EC - 1),
                    )
                tt = tp.tile([128, 512], bf16, tag="tt",
                             name=f"tt_{b}_{st}_{ac}", bufs=3)
                nc.scalar.activation(
                    tt[:], pa[:], Act.Tanh,
                    bias=bias_sb[:, ac * BL + b:ac * BL + b + 1],
                )
                nc.tensor.matmul(
                    ps[:], v_sb[:, ac:ac + 1], tt[:],
                    start=(ac == 0), stop=(ac == AC - 1),
                )
            nc.scalar.activation(
                p_rows[b][:, st * 512:(st + 1) * 512], ps[:], Act.Exp)
            # mask + bf16 cast for this s-slice, so the context tile for
            # this slice can start without waiting for the whole batch
            sl = slice(st * 512, (st + 1) * 512)
            nc.vector.tensor_tensor(p_rows[b][:, sl], p_rows[b][:, sl],
                                    mask_rows[b][:, sl], Alu.mult)
            nc.vector.tensor_copy(pbf_rows[b][:, sl], p_rows[b][:, sl])

        def emit_softmax(b):
            p_row = p_rows[b]
            nc.vector.tensor_reduce(d_cols[:, b:b + 1], p_row[:], AX.X,
                                    Alu.add)
            nc.vector.reciprocal(r_cols[:, b:b + 1], d_cols[:, b:b + 1])
            # normalize in place and write the weights row out
            nc.vector.tensor_scalar_mul(p_row[:], p_row[:],
                                        r_cols[:, b:b + 1])
            nc.sync.dma_start(weights.ap()[b:b + 1, :], p_row[:])

        def emit_ctx_tile(b, st):
            # broadcast p[st] to 128 partitions via K=1 ones-matmul
            pbc = psB.tile([128, 512], f32, tag="pbc", name=f"pbc_{b}_{st}")
            nc.tensor.matmul(pbc[:], ones_bf[:],
                             pbf_rows[b][:, st * 512:(st + 1) * 512],
                             start=True, stop=True)
            pbc_sb = tp.tile([128, 512], bf16, tag="pbcsb",
                             name=f"pbcsb_{b}_{st}", bufs=2)
            nc.scalar.copy(pbc_sb[:], pbc[:])
            scr = tp.tile([128, EC, 512], bf16, tag="scr",
                          name=f"scr_{b}_{st}", bufs=2)
            nc.vector.tensor_tensor(
                scr[:], enc_tiles[b][:, :, st * 512:(st + 1) * 512],
                bcast3(pbc_sb[:]), Alu.mult)
            # partial sums over this s-slice: [128, EC, 512] -> [128, EC]
            dst = ctx_sts[b][:, st * EC:(st + 1) * EC]
            if b == BL - 1:
                # drain phase: split the reduce across both engines
                nc.vector.tensor_reduce(
                    ctx_sts[b][:, st * EC:st * EC + EC // 2],
                    scr[:, :EC // 2, :], AX.X, Alu.add)
                for ec in range(EC // 2, EC):
                    nc.scalar.activation(
                        scr[:, ec, :], scr[:, ec, :], Act.Copy,
                        accum_out=ctx_sts[b][:, st * EC + ec:st * EC + ec + 1])
            elif st % 4 != 3:
                nc.vector.tensor_reduce(dst, scr[:], AX.X, Alu.add)
            else:
                for ec in range(EC):
                    nc.scalar.activation(
                        scr[:, ec, :], scr[:, ec, :], Act.Copy,
                        accum_out=ctx_sts[b][:, st * EC + ec:st * EC + ec + 1])

        def emit_ctx_final(b):
            # sum partials over s-tiles: view [128, ST, EC] as [128, EC, ST]
            nc.vector.tensor_reduce(
                ctx_cols[:, b * EC:(b + 1) * EC],
                ctx_sts[b].rearrange("p (s e) -> p e s", e=EC),
                AX.X, Alu.add)

        def begin_batch(b, after_first=None):
            emit_load(b, after_first)
            mask_row = tp.tile([1, S], bf16, tag="mask", name=f"mask_{b}",
                               bufs=1)
            nc.sync.dma_start(mask_row[:], maskb.ap()[b:b + 1, :])
            p_row = tp.tile([1, S], f32, tag="prow", name=f"p_{b}", bufs=2)
            p_rows[b] = p_row
            mask_rows[b] = mask_row
            pbf_rows[b] = tp.tile([1, S], bf16, tag="pbf", name=f"pbf_{b}",
                                  bufs=1)
            ctx_sts[b] = tp.tile([128, ST * EC], f32, tag="ctxst",
                                 name=f"ctxst_{b}", bufs=2)

        # --- main loop, software-pipelined one s-tile deep ---
        # flat list of (b, st) work items; ctx lags scores by one item
        items = [(b, st) for b in range(BL) for st in range(ST)]
        nc.sync.dma_start(v_sb[:], vt.ap())
        begin_batch(0, after_first=lambda: nc.sync.dma_start(
            wh_sb[:], wh.ap().rearrange("(c p) a -> p c a", p=128)))
        nc.sync.dma_start(ident_sb[:], ident.ap())
        # decoder projection: bias[a, b] = sum_d W_s[d, a] * ds[b, d]
        pd = psB.tile([128, AC * BL], f32, tag="pbc", name="pd")
        for ac in range(AC):
            for ec in range(EC):
                nc.tensor.matmul(
                    pd[:, ac * BL:(ac + 1) * BL],
                    ws_sb[:, ec, ac * 128:(ac + 1) * 128],
                    dsT_sb[:, ec, :],
                    start=(ec == 0), stop=(ec == EC - 1),
                )
        nc.scalar.copy(bias_sb[:], pd[:])
        for i, (b, st) in enumerate(items):
            emit_scores_tile(b, st)
            if st == 0 and b + 1 < BL:
                begin_batch(b + 1)
            if i > 0:
                pb, pst = items[i - 1]
                emit_ctx_tile(pb, pst)
                if pst == ST - 1:
                    emit_ctx_final(pb)
                    emit_softmax(pb)
        lb, lst = items[-1]
        emit_softmax(lb)
        emit_ctx_tile(lb, lst)
        emit_ctx_final(lb)

        # --- normalize context, transpose on PE, write out contiguously ---
        pr = psB.tile([128, BL], f32, tag="pbc", name="pr")
        nc.tensor.matmul(pr[:], ones_f32[:], r_cols[:], start=True, stop=True)
        rrep = tp.tile([128, BL], f32, tag="rrep", name="rrep", bufs=1)
        nc.scalar.copy(rrep[:], pr[:])
        ctx_fin = tp.tile([128, BL * EC], f32, tag="ctxfin", name="ctx_fin",
                          bufs=1)
        for b in range(BL):
            nc.vector.tensor_scalar_mul(ctx_fin[:, b * EC:(b + 1) * EC],
                                        ctx_cols[:, b * EC:(b + 1) * EC],
                                        rrep[:, b:b + 1])
        ctxT_ps = psA.tile([BL * EC, 128], f32, tag="pa", name="ctxT_ps")
        nc.tensor.transpose(ctxT_ps[:], ctx_fin[:], ident_sb[:])
        ctxT_sb = tp.tile([BL * EC, 128], f32, tag="ctxT", name="ctxT_sb",
                          bufs=1)
        nc.scalar.copy(ctxT_sb[:], ctxT_ps[:])
        nc.sync.dma_start(
            context.ap().rearrange("b (c p) -> (b c) p", p=128), ctxT_sb[:])

    nc.compile()
    _NC_CACHE = nc
    return nc


def kernel(decoder_state, encoder_outputs, mask, W_h, W_s, v):
    nc = _build()
    from concourse.bass_utils import run_bass_kernel_spmd

    enc_bf = np.asarray(encoder_outputs, dtype=np.float32).astype(BF16)
    # [B, S, E] -> [B, E, S] -> [B, EC, 128, S]
    encT_all = np.ascontiguousarray(enc_bf.transpose(0, 2, 1)).reshape(
        B, EC, 128, S)
    wh_bf = np.asarray(W_h, dtype=np.float32).astype(BF16)
    ws_bf = np.asarray(W_s, dtype=np.float32).astype(BF16)
    ds = np.asarray(decoder_state, dtype=np.float32)
    v_t = np.ascontiguousarray(
        np.asarray(v, dtype=np.float32).reshape(AC, 128).T).astype(BF16)
    mask_bf = np.asarray(mask).astype(BF16)
    ident = np.eye(128, dtype=np.float32)

    in_maps = []
    for i in range(NCORES):
        sl = slice(i * BL, (i + 1) * BL)
        in_maps.append({
            "encT": encT_all[sl],
            "wh": wh_bf,
            "ws": ws_bf,
            "dsT": np.ascontiguousarray(ds[sl].T).astype(BF16),
            "vt": v_t,
            "maskb": mask_bf[sl],
            "ident": ident,
        })

    res = run_bass_kernel_spmd(nc, in_maps, core_ids=list(range(NCORES)))
    ctx = np.concatenate(
        [np.asarray(res.results[i]["context"]) for i in range(NCORES)], axis=0)
    wts = np.concatenate(
        [np.asarray(res.results[i]["weights"]) for i in range(NCORES)], axis=0)
    return ctx.astype(np.float32), wts.astype(np.float32)
